# revision 2
# baseline (speedup 1.0000x reference)
"""Trainium2 Bass kernel for nn_CNN2DImplemented_51994874085714.

conv2d: x (16, 64, 112, 112) f32 * weight (64, 3, 3, 128) -> (16, 128, 112, 112),
3x3, pad=1, stride=1 (weight layout (C_in, kh, kw, C_out), no bias).

Sharding: data-parallel over batch - 2 images per NeuronCore on 8 cores,
weight replicated; each core computes its shard independently (no
collectives) and the host concatenates the per-core outputs.

Per-core kernel (implicit GEMM):
  out[b, o, h, w] = sum_{c, dh, dw} x_pad[b, c, h+dh, w+dw] * weight[c, dh, dw, o]

x and weight DRAM tensors are declared float32r (same bits as f32): the PE
runs fp32r matmuls at 1 column/cycle (4x the fp32 rate) at ~1.6e-4 relative
accuracy. SBUF holds x in 28-output-row strips as [128, S+3, W+2] tiles:
partitions 0:64 are x_pad rows h0..h0+S+2 (block A, DMA'd directly),
partitions 64:128 the same rows shifted down one (block B, produced by a
GPSIMD on-chip move). One K=128 matmul then contracts channel taps
(dh, dh+1) together:
  pair MM  (dh=0,1): lhsT = [W[:,0,dw,:]; W[:,1,dw,:]]
  single MM (dh=2):  lhsT = [W[:,2,dw,:]; 0]
Six matmuls of N=4*W=448 accumulate each PSUM bank ([O=128, 4, W]); DVE
copies banks to an SBUF staging strip which is stored with one DMA.
"""

from contextlib import ExitStack

import numpy as np

N_CORES = 8
B, C, H, W, O = 16, 64, 112, 112, 128
B_LOC = B // N_CORES
S = 28  # output rows per strip

_cache = {}


def _build_nc():
    import concourse.mybir as mybir
    import concourse.tile as tile
    from concourse import bacc

    F32 = mybir.dt.float32
    F32R = mybir.dt.float32r

    nc = bacc.Bacc("TRN2", target_bir_lowering=False, debug=False,
                   num_devices=N_CORES)
    x_d = nc.declare_dram_parameter("x", [B_LOC, C, H, W], F32R, isOutput=False)
    w_d = nc.declare_dram_parameter("weight", [C, 3, 3, O], F32R, isOutput=False)
    o_d = nc.declare_dram_parameter("out", [B_LOC, O, H, W], F32, isOutput=True)

    R = S + 3
    Wp = W + 2
    NS = H // S

    with tile.TileContext(nc) as tc, ExitStack() as ctx:
        wpool = ctx.enter_context(tc.tile_pool(name="weights", bufs=1))
        xpool = ctx.enter_context(tc.tile_pool(name="xstrips", bufs=4))
        spool = ctx.enter_context(tc.tile_pool(name="staging", bufs=3))
        ppool = ctx.enter_context(tc.tile_pool(name="psum", bufs=6, space="PSUM"))

        zrow = wpool.tile([64, O], F32, tag="zrow")
        nc.vector.memset(zrow[:, :], 0.0)
        wpair = []
        wsing = []
        for dw in range(3):
            wp = wpool.tile([128, O], F32R, tag=f"wpair{dw}")
            ws = wpool.tile([128, O], F32R, tag=f"wsing{dw}")
            nc.sync.dma_start(wp[0:64, :], w_d[:, 0, dw, :])
            nc.sync.dma_start(wp[64:128, :], w_d[:, 1, dw, :])
            nc.sync.dma_start(ws[0:64, :], w_d[:, 2, dw, :])
            nc.vector.tensor_copy(ws[64:128, :], zrow[:, :])
            wpair.append(wp)
            wsing.append(ws)

        def load_task(t):
            s, img = t
            h0 = s * S
            xb = xpool.tile([128, R, Wp], F32R, tag="xs")
            xbf = xb.bitcast(F32)
            nc.vector.memset(xbf[0:64, :, 0], 0.0)
            nc.vector.memset(xbf[0:64, :, Wp - 1], 0.0)
            r_lo = max(0, 1 - h0)
            r_hi = min(S + 2, H - h0)
            if r_lo > 0:
                nc.vector.memset(xbf[0:64, 0:r_lo, :], 0.0)
            if r_hi < S + 2:
                nc.vector.memset(xbf[0:64, r_hi + 1:S + 3, :], 0.0)
            nc.sync.dma_start(
                xb[0:64, r_lo:r_hi + 1, 1:W + 1],
                x_d[img, :, h0 + r_lo - 1:h0 + r_hi, :],
            )
            nc.gpsimd.tensor_copy(xb[64:128, 0:S + 2, :], xb[0:64, 1:S + 3, :])
            return xb

        def compute(s, img, xb):
            h0 = s * S
            stg = spool.tile([O, S, W], F32, tag="stg")
            for j in range(S // 4):
                l0 = 4 * j
                ps = ppool.tile([O, 4, W], F32, tag="ps")
                for dw in range(3):
                    nc.tensor.matmul(
                        ps[:, :, :],
                        wpair[dw][:, :],
                        xb[:, l0:l0 + 4, dw:dw + W],
                        start=(dw == 0), stop=False,
                    )
                for dw in range(3):
                    nc.tensor.matmul(
                        ps[:, :, :],
                        wsing[dw][:, :],
                        xb[:, l0 + 2:l0 + 6, dw:dw + W],
                        start=False, stop=(dw == 2),
                    )
                nc.vector.tensor_copy(stg[:, l0:l0 + 4, :], ps[:, :, :])
            nc.sync.dma_start(o_d[img, :, h0:h0 + S, :], stg[:, :, :])

        tasks = [(s, img) for s in range(NS) for img in range(B_LOC)]
        cur = load_task(tasks[0])
        for i, t in enumerate(tasks):
            nxt = load_task(tasks[i + 1]) if i + 1 < len(tasks) else None
            compute(t[0], t[1], cur)
            cur = nxt

    nc.compile()
    return nc


def _make_in_maps(x, weight):
    x = np.ascontiguousarray(np.asarray(x, dtype=np.float32))
    w = np.ascontiguousarray(np.asarray(weight, dtype=np.float32))
    return [
        {"x": x[i * B_LOC:(i + 1) * B_LOC], "weight": w} for i in range(N_CORES)
    ]


def kernel(x: np.ndarray, weight: np.ndarray) -> np.ndarray:
    from concourse.bass_utils import run_bass_kernel_spmd

    if "nc" not in _cache:
        _cache["nc"] = _build_nc()
    nc = _cache["nc"]

    in_maps = _make_in_maps(x, weight)
    res = run_bass_kernel_spmd(nc, in_maps, list(range(N_CORES)))
    return np.concatenate(
        [res.results[i]["out"] for i in range(N_CORES)], axis=0
    )



# revision 4
# speedup vs baseline: 454.3504x; 454.3504x over previous
"""Trainium2 Bass kernel for nn_CNN2DImplemented_51994874085714.

conv2d: x (16, 64, 112, 112) f32 * weight (64, 3, 3, 128) -> (16, 128, 112, 112),
3x3, pad=1, stride=1 (weight layout (C_in, kh, kw, C_out), no bias).

Sharding: data-parallel over batch - 2 images per NeuronCore on 8 cores,
weight replicated; each core computes its shard independently (no
collectives) and the host concatenates the per-core outputs.

Per-core kernel (implicit GEMM, bf16 data / f32 PSUM accumulation; the host
casts f32 -> bf16 on the way in and bf16 -> f32 on the way out, which is well
inside the 2e-2 relative-error budget):

Per 4-output-row PSUM group, 5 full-width (N=448) K=128 matmuls:
  T1 tile [A; B]  A[c,r]=x[h0+r-1], B[c,r]=x[h0+r]  (r=0..Sl):
    MM1 pair (0,1)+(1,1): rhs T1[:, l0:l0+4, :]
    MM2 single (2,1):     rhs T1[:, l0+1:l0+5, :]  lhsT=[0; W(2,1)]
  T2 tile [C; D]  C[c,r,w]=x[h0+r-1, w-1], D[c,r,w]=x[h0+r-1, w+1] (r=0..Sl+1):
    MM3-5 pair (dh,0)+(dh,2): rhs T2[:, l0+dh:l0+dh+4, :]
T1 is loaded as one dual-half DMA (row-shifted halves, contiguous per-partition
descriptors); T2 as one dual-half DMA of column-shifted flat windows of the
image (edge columns fixed by memsets after the DMA). A short stream of dummy
warmup matmuls keeps the PE busy (and its clock ramp going) while the first
strip's tiles load; strip heights ramp 8..32 at the start (fast first load ->
early PE start) and down to 8 at the end (small tail store). PSUM->SBUF
evacuation rotates over DVE/Activation/GpSimd so no single engine gates the
PE's PSUM-bank recycling.
"""

from contextlib import ExitStack

import numpy as np

N_CORES = 8
B, C, H, W, O = 16, 64, 112, 112, 128
B_LOC = B // N_CORES

_cache = {}


def _task_list(B_loc, H):
    tasks = []
    for img in range(B_loc):
        if img == 0:
            ss = [8, 16, 24, 32, 32]   # ramp up: fast first loads
        elif img == B_loc - 1:
            ss = [28, 28, 28, 20, 8]   # ramp down: small tail store
        else:
            ss = [28, 28, 28, 28]
        h0 = 0
        for sl in ss:
            tasks.append((img, h0, sl))
            h0 += sl
        assert h0 == H
    return tasks


def _build_nc(B_loc=B_LOC, lookahead=4, n_warmup=35):
    import concourse.mybir as mybir
    import concourse.tile as tile
    from concourse import bacc
    from concourse.bass import AP

    BF16 = mybir.dt.bfloat16
    F32 = mybir.dt.float32

    HW_ = H * W

    nc = bacc.Bacc("TRN2", target_bir_lowering=False, debug=False,
                   num_devices=N_CORES)
    x_d = nc.declare_dram_parameter("x", [B_loc, C, H, W], BF16, isOutput=False)
    w_d = nc.declare_dram_parameter("weight", [C, 3, 3, O], BF16, isOutput=False)
    o_d = nc.declare_dram_parameter("out", [B_loc, O, H, W], BF16, isOutput=True)

    tasks = _task_list(B_loc, H)
    slmax = max(t[2] for t in tasks)

    with tile.TileContext(nc) as tc, ExitStack() as ctx:
        wpool = ctx.enter_context(tc.tile_pool(name="weights", bufs=1))
        t1pool = ctx.enter_context(tc.tile_pool(name="t1", bufs=5))
        t2pool = ctx.enter_context(tc.tile_pool(name="t2", bufs=5))
        spool = ctx.enter_context(tc.tile_pool(name="staging", bufs=3))
        ppool = ctx.enter_context(tc.tile_pool(name="psum", bufs=8, space="PSUM"))

        # ---- PE warmup: dummy matmuls on zeroed scratch while loads run ----
        scr_w = wpool.tile([128, O], BF16, tag="scr_w")
        scr_x = wpool.tile([128, W], BF16, tag="scr_x")
        ps_warm = ppool.tile([O, 4, W], F32, tag="ps")
        nc.vector.memset(scr_w[:, :], 0.0)
        nc.vector.memset(scr_x[:, :], 0.0)
        for _ in range(n_warmup):
            nc.tensor.matmul(ps_warm[:, 0, :], scr_w[:, :], scr_x[:, :],
                             start=True, stop=True)

        # ---- weights ----
        # One DMA for the whole weight, then split into the 5 lhsT tiles
        # on-chip: same-partition halves on DVE, cross-partition on gpsimd.
        wraw = wpool.tile([64, 9, O], BF16, tag="wraw")
        p01c = wpool.tile([128, O], BF16, tag="p01c")
        s21 = wpool.tile([128, O], BF16, tag="s21")
        p02 = [wpool.tile([128, O], BF16, tag=f"p02_{dh}", name=f"p02_{dh}")
               for dh in range(3)]

        def load_weights():
            # wraw[c, dh*3+dw, o] = w[c, dh, dw, o]
            nc.sync.dma_start(wraw[:, :, :], w_d[:, :, :, :])
            # tops (partitions 0:64), same-partition -> DVE
            nc.vector.tensor_copy(p01c[0:64, :], wraw[:, 1, :])   # W(0,1)
            nc.vector.memset(s21[0:64, :], 0.0)
            for dh in range(3):
                nc.vector.tensor_copy(p02[dh][0:64, :], wraw[:, 3 * dh, :])
            # bottoms (partitions 64:128), cross-partition -> gpsimd
            nc.gpsimd.tensor_copy(p01c[64:128, :], wraw[:, 4, :])  # W(1,1)
            nc.gpsimd.tensor_copy(s21[64:128, :], wraw[:, 7, :])   # W(2,1)
            for dh in range(3):
                nc.gpsimd.tensor_copy(p02[dh][64:128, :], wraw[:, 3 * dh + 2, :])

        def load_task(t):
            img, h0, sl = t
            r1 = sl + 1
            r2 = sl + 2
            fl2 = r2 * W
            t1 = t1pool.tile([128, slmax + 1, W], BF16, tag="t1",
                             name="t1")[:, 0:r1, :]
            t2 = t2pool.tile([128, slmax + 2, W], BF16, tag="t2",
                             name="t2")[:, 0:r2, :]
            t2f = t2.rearrange("p a b -> p (a b)")
            xi = x_d[img]  # (C, H, W)
            xif = xi.rearrange("c a b -> c (a b)")

            # ---- T1: A half rows r=0..sl <- x rows h0-1..h0+sl-1
            #          B half rows r=0..sl <- x rows h0..h0+sl
            if h0 == 0:
                nc.sync.dma_start(t1[0:64, 1:r1, :], xi[:, 0:sl, :])
                nc.sync.dma_start(t1[64:128, 0:r1, :], xi[:, 0:r1, :])
                nc.vector.memset(t1[0:64, 0:1, :], 0.0)
            elif h0 + sl == H:
                nc.sync.dma_start(t1[0:64, 0:r1, :], xi[:, h0 - 1:h0 + sl, :])
                nc.sync.dma_start(t1[64:128, 0:sl, :], xi[:, h0:h0 + sl, :])
                nc.vector.memset(t1[64:128, sl:r1, :], 0.0)
            else:
                # dual-half: partition p = half*64 + c ; half offset = +W
                nc.sync.dma_start(
                    t1[:, :, :],
                    AP(x_d, img * C * HW_ + (h0 - 1) * W,
                       [[W, 2], [HW_, C], [W, r1], [1, W]]))

            # ---- T2: C half flat <- x flat[(h0-1)*W - 1 : +fl2]
            #          D half flat <- x flat[(h0-1)*W + 1 : +fl2]
            base_c = (h0 - 1) * W - 1
            base_d = (h0 - 1) * W + 1
            if h0 == 0:
                nc.sync.dma_start(t2f[0:64, W + 1:fl2], xif[:, 0:fl2 - W - 1])
                nc.sync.dma_start(t2f[64:128, W:fl2], xif[:, 1:fl2 - W + 1])
                nc.gpsimd.memset(t2[:, 0:1, :], 0.0)  # row 0 both halves
            elif h0 + sl == H:
                nc.sync.dma_start(
                    t2f[0:64, 0:(r2 - 1) * W],
                    xif[:, base_c:base_c + (r2 - 1) * W])
                nc.sync.dma_start(
                    t2f[64:128, 0:(r2 - 1) * W - 1],
                    xif[:, base_d:base_d + (r2 - 1) * W - 1])
                nc.gpsimd.memset(t2[:, r2 - 1:r2, :], 0.0)  # last row
            else:
                nc.sync.dma_start(
                    t2f[:, :],
                    AP(x_d, img * C * HW_ + base_c,
                       [[2, 2], [HW_, C], [1, fl2]]))
            # edge columns (garbage from flat wrap) -> zero, after the DMA
            nc.gpsimd.memset(t2[0:64, :, 0:1], 0.0)
            nc.gpsimd.memset(t2[64:128, :, W - 1:W], 0.0)
            return t1, t2

        copy_rot = [
            nc.vector.tensor_copy,
            nc.scalar.copy,
        ]

        def compute(t, tiles):
            img, h0, sl = t
            t1, t2 = tiles
            stg = spool.tile([O, slmax, W], BF16, tag="stg",
                             name="stg")[:, 0:sl, :]
            for j in range(sl // 4):
                l0 = 4 * j
                ps = ppool.tile([O, 4, W], F32, tag="ps")
                nc.tensor.matmul(ps[:, :, :], p01c[:, :],
                                 t1[:, l0:l0 + 4, :], start=True, stop=False)
                nc.tensor.matmul(ps[:, :, :], s21[:, :],
                                 t1[:, l0 + 1:l0 + 5, :], start=False,
                                 stop=False)
                for dh in range(3):
                    nc.tensor.matmul(ps[:, :, :], p02[dh][:, :],
                                     t2[:, l0 + dh:l0 + dh + 4, :],
                                     start=False, stop=(dh == 2))
                eng = copy_rot[j % len(copy_rot)]
                eng(stg[:, l0:l0 + 4, :], ps[:, :, :])
            nc.sync.dma_start(o_d[img, :, h0:h0 + sl, :], stg[:, :, :])

        n = len(tasks)
        tiles = {}
        load_weights()
        tiles[0] = load_task(tasks[0])
        for i in range(1, min(lookahead, n)):
            tiles[i] = load_task(tasks[i])
        for i in range(n):
            compute(tasks[i], tiles.pop(i))
            if i + lookahead < n:
                tiles[i + lookahead] = load_task(tasks[i + lookahead])

    nc.compile()
    return nc


def _make_in_maps(x, weight):
    import ml_dtypes

    bf16 = ml_dtypes.bfloat16
    x = np.ascontiguousarray(np.asarray(x).astype(bf16))
    w = np.ascontiguousarray(np.asarray(weight).astype(bf16))
    return [
        {"x": x[i * B_LOC:(i + 1) * B_LOC], "weight": w} for i in range(N_CORES)
    ]


def kernel(x: np.ndarray, weight: np.ndarray) -> np.ndarray:
    from concourse.bass_utils import run_bass_kernel_spmd

    if "nc" not in _cache:
        _cache["nc"] = _build_nc()
    nc = _cache["nc"]

    in_maps = _make_in_maps(x, weight)
    res = run_bass_kernel_spmd(nc, in_maps, list(range(N_CORES)))
    out = np.concatenate(
        [np.asarray(res.results[i]["out"]) for i in range(N_CORES)], axis=0
    )
    return out.astype(np.float32)


# revision 5
# speedup vs baseline: 458.0662x; 1.0082x over previous
"""Trainium2 Bass kernel for nn_CNN2DImplemented_51994874085714.

conv2d: x (16, 64, 112, 112) f32 * weight (64, 3, 3, 128) -> (16, 128, 112, 112),
3x3, pad=1, stride=1 (weight layout (C_in, kh, kw, C_out), no bias).

Sharding: data-parallel over batch - 2 images per NeuronCore on 8 cores,
weight replicated; each core computes its shard independently (no
collectives) and the host concatenates the per-core outputs.

Per-core kernel: implicit GEMM in bf16 with f32 PSUM accumulation (the host
casts f32 -> bf16 on the way in and bf16 -> f32 on the way out; well inside
the 2e-2 relative-error budget). The host also ships x H-padded and
flattened per channel:
  x_dev[b, c, :] = [0] ++ flat(zeros(1,W) ++ x[b,c] ++ zeros(1,W)) ++ [0]
(FL = (H+2)*W + 2; the guard elements keep the column-shifted windows below
in bounds, and padded zero rows make every strip load uniform).

Per 4-output-row PSUM group, 5 full-width (N=448) K=128 matmuls:
  T1 tile [A; B]  A[c,r]=x[h0+r-1], B[c,r]=x[h0+r]  (r=0..Sl):
    MM1 pair (0,1)+(1,1): rhs T1[:, l0:l0+4, :]
    MM2 single (2,1):     rhs T1[:, l0+1:l0+5, :]  lhsT=[0; W(2,1)]
  T2 tile [C; D]  C[c,r,w]=x[h0+r-1, w-1], D[c,r,w]=x[h0+r-1, w+1] (r=0..Sl+1):
    MM3-5 pair (dh,0)+(dh,2): rhs T2[:, l0+dh:l0+dh+4, :]
Every strip loads with exactly 2 dual-half DMAs (row-shifted halves for T1,
column-shifted flat windows for T2; contiguous per-partition descriptors at
full DMA bandwidth); the only fixups are two per-strip edge-column memsets
(flat-window wrap garbage). A short stream of dummy warmup matmuls keeps the
PE busy (and its clock ramp going) while the first strip loads; strip
heights ramp 8..32 at the start (fast first load -> early PE start) and down
to 4 at the end (small tail store). PSUM->SBUF evacuation alternates between
DVE and the Activation engine so neither gates PSUM-bank recycling, and x
loads run 4 strips ahead.
"""

from contextlib import ExitStack

import numpy as np

N_CORES = 8
B, C, H, W, O = 16, 64, 112, 112, 128
B_LOC = B // N_CORES
FL = (H + 2) * W + 2  # padded flat length per channel

_cache = {}


def _task_list(B_loc, H):
    tasks = []
    for img in range(B_loc):
        if img == 0:
            ss = [8, 16, 24, 32, 32]   # ramp up: fast first loads
        elif img == B_loc - 1:
            ss = [28, 28, 28, 24, 4]   # ramp down: small tail store
        else:
            ss = [28, 28, 28, 28]
        h0 = 0
        for sl in ss:
            tasks.append((img, h0, sl))
            h0 += sl
        assert h0 == H
    return tasks


def _build_nc(B_loc=B_LOC, lookahead=4, n_warmup=35):
    import concourse.mybir as mybir
    import concourse.tile as tile
    from concourse import bacc
    from concourse.bass import AP

    BF16 = mybir.dt.bfloat16
    F32 = mybir.dt.float32

    nc = bacc.Bacc("TRN2", target_bir_lowering=False, debug=False,
                   num_devices=N_CORES)
    x_d = nc.declare_dram_parameter("x", [B_loc, C, FL], BF16, isOutput=False)
    w_d = nc.declare_dram_parameter("weight", [C, 3, 3, O], BF16, isOutput=False)
    o_d = nc.declare_dram_parameter("out", [B_loc, O, H, W], BF16, isOutput=True)

    tasks = _task_list(B_loc, H)
    slmax = max(t[2] for t in tasks)

    with tile.TileContext(nc) as tc, ExitStack() as ctx:
        wpool = ctx.enter_context(tc.tile_pool(name="weights", bufs=1))
        t1pool = ctx.enter_context(tc.tile_pool(name="t1", bufs=5))
        t2pool = ctx.enter_context(tc.tile_pool(name="t2", bufs=5))
        spool = ctx.enter_context(tc.tile_pool(name="staging", bufs=4))
        ppool = ctx.enter_context(tc.tile_pool(name="psum", bufs=8, space="PSUM"))

        # ---- PE warmup: dummy matmuls on zeroed scratch while loads run ----
        scr_w = wpool.tile([128, O], BF16, tag="scr_w")
        scr_x = wpool.tile([128, W], BF16, tag="scr_x")
        ps_warm = ppool.tile([O, 4, W], F32, tag="ps")
        nc.gpsimd.memset(scr_w[:, :], 0.0)
        nc.gpsimd.memset(scr_x[:, :], 0.0)
        for _ in range(n_warmup):
            nc.tensor.matmul(ps_warm[:, 0, :], scr_w[:, :], scr_x[:, :],
                             start=True, stop=True)

        # ---- weights ----
        # One DMA for the whole weight, then split into the 5 lhsT tiles
        # on-chip with DVE copies.
        wraw = wpool.tile([64, 9, O], BF16, tag="wraw")
        p01c = wpool.tile([128, O], BF16, tag="p01c")
        s21 = wpool.tile([128, O], BF16, tag="s21")
        p02 = [wpool.tile([128, O], BF16, tag=f"p02_{dh}", name=f"p02_{dh}")
               for dh in range(3)]

        def load_weights():
            # wraw[c, dh*3+dw, o] = w[c, dh, dw, o]
            nc.sync.dma_start(wraw[:, :, :], w_d[:, :, :, :])
            nc.vector.tensor_copy(p01c[0:64, :], wraw[:, 1, :])    # W(0,1)
            nc.vector.tensor_copy(p01c[64:128, :], wraw[:, 4, :])  # W(1,1)
            nc.vector.memset(s21[0:64, :], 0.0)
            nc.vector.tensor_copy(s21[64:128, :], wraw[:, 7, :])   # W(2,1)
            for dh in range(3):
                nc.vector.tensor_copy(p02[dh][0:64, :], wraw[:, 3 * dh, :])
                nc.vector.tensor_copy(p02[dh][64:128, :],
                                      wraw[:, 3 * dh + 2, :])

        def load_task(t):
            img, h0, sl = t
            r1 = sl + 1
            r2 = sl + 2
            fl2 = r2 * W
            t1 = t1pool.tile([128, slmax + 1, W], BF16, tag="t1",
                             name="t1")[:, 0:r1, :]
            t2 = t2pool.tile([128, slmax + 2, W], BF16, tag="t2",
                             name="t2")[:, 0:r2, :]
            t2f = t2.rearrange("p a b -> p (a b)")
            base = img * C * FL + 1 + h0 * W  # flat addr of padded row h0

            # T1 halves: A rows <- padded rows h0..h0+sl (= x rows h0-1..),
            #            B rows <- padded rows h0+1..h0+sl+1 (shift dim +W)
            nc.sync.dma_start(
                t1[:, :, :],
                AP(x_d, base, [[W, 2], [FL, C], [W, r1], [1, W]]))
            # T2 halves: C flat <- [base-1 : +fl2], D flat <- [base+1 : +fl2]
            nc.sync.dma_start(
                t2f[:, :],
                AP(x_d, base - 1, [[2, 2], [FL, C], [1, fl2]]))
            # edge columns (garbage from flat wrap) -> zero, after the DMA
            nc.gpsimd.memset(t2[0:64, :, 0:1], 0.0)
            nc.gpsimd.memset(t2[64:128, :, W - 1:W], 0.0)
            return t1, t2

        def compute(t, tiles):
            img, h0, sl = t
            t1, t2 = tiles
            stg = spool.tile([O, slmax, W], BF16, tag="stg",
                             name="stg")[:, 0:sl, :]
            for j in range(sl // 4):
                l0 = 4 * j
                ps = ppool.tile([O, 4, W], F32, tag="ps")
                nc.tensor.matmul(ps[:, :, :], p01c[:, :],
                                 t1[:, l0:l0 + 4, :], start=True, stop=False)
                nc.tensor.matmul(ps[:, :, :], s21[:, :],
                                 t1[:, l0 + 1:l0 + 5, :], start=False,
                                 stop=False)
                for dh in range(3):
                    nc.tensor.matmul(ps[:, :, :], p02[dh][:, :],
                                     t2[:, l0 + dh:l0 + dh + 4, :],
                                     start=False, stop=(dh == 2))
                if j % 2 == 0:
                    nc.vector.tensor_copy(stg[:, l0:l0 + 4, :], ps[:, :, :])
                else:
                    nc.scalar.copy(stg[:, l0:l0 + 4, :], ps[:, :, :])
            nc.sync.dma_start(o_d[img, :, h0:h0 + sl, :], stg[:, :, :])

        n = len(tasks)
        tiles = {}
        load_weights()
        tiles[0] = load_task(tasks[0])
        for i in range(1, min(lookahead, n)):
            tiles[i] = load_task(tasks[i])
        for i in range(n):
            compute(tasks[i], tiles.pop(i))
            if i + lookahead < n:
                tiles[i + lookahead] = load_task(tasks[i + lookahead])

    nc.compile()
    return nc


def _pad_x_flat(x):
    """(B, C, H, W) -> (B, C, FL) H-padded flat layout (see module doc)."""
    Bn, Cn, Hn, Wn = x.shape
    xp = np.zeros((Bn, Cn, FL), dtype=x.dtype)
    xp[:, :, 1 + Wn:1 + Wn + Hn * Wn] = x.reshape(Bn, Cn, Hn * Wn)
    return xp


def _make_in_maps(x, weight):
    import ml_dtypes

    bf16 = ml_dtypes.bfloat16
    xp = _pad_x_flat(np.ascontiguousarray(np.asarray(x).astype(bf16)))
    w = np.ascontiguousarray(np.asarray(weight).astype(bf16))
    return [
        {"x": xp[i * B_LOC:(i + 1) * B_LOC], "weight": w}
        for i in range(N_CORES)
    ]


def kernel(x: np.ndarray, weight: np.ndarray) -> np.ndarray:
    from concourse.bass_utils import run_bass_kernel_spmd

    if "nc" not in _cache:
        _cache["nc"] = _build_nc()
    nc = _cache["nc"]

    in_maps = _make_in_maps(x, weight)
    res = run_bass_kernel_spmd(nc, in_maps, list(range(N_CORES)))
    out = np.concatenate(
        [np.asarray(res.results[i]["out"]) for i in range(N_CORES)], axis=0
    )
    return out.astype(np.float32)


# revision 6
# speedup vs baseline: 463.7211x; 1.0123x over previous
"""Trainium2 Bass kernel for nn_CNN2DImplemented_51994874085714.

conv2d: x (16, 64, 112, 112) f32 * weight (64, 3, 3, 128) -> (16, 128, 112, 112),
3x3, pad=1, stride=1 (weight layout (C_in, kh, kw, C_out), no bias).

Sharding: data-parallel over batch - 2 images per NeuronCore on 8 cores,
weight replicated; each core computes its shard independently (no
collectives) and the host concatenates the per-core outputs.

Per-core kernel: implicit GEMM in bf16 with f32 PSUM accumulation (the host
casts f32 -> bf16 on the way in and bf16 -> f32 on the way out; well inside
the 2e-2 relative-error budget). The host also ships x H-padded and
flattened per channel:
  x_dev[b, c, :] = [0] ++ flat(114 rows of 113: [row | 0]) ++ [0]
(FL = (H+2)*(W+1) + 2: zero pad rows above/below, one zero separator after
each row, one guard element at each end). The separators land exactly where
the +-1-column-shifted windows below would otherwise wrap garbage, so the
conv's zero padding falls out of the layout with no on-chip fixups.

Per 4-output-row PSUM group, 5 full-width (N=448) K=128 matmuls:
  T1 tile [A; B]  A[c,r]=x[h0+r-1], B[c,r]=x[h0+r]  (r=0..Sl):
    MM1 pair (0,1)+(1,1): rhs T1[:, l0:l0+4, :]
    MM2 single (2,1):     rhs T1[:, l0+1:l0+5, :]  lhsT=[0; W(2,1)]
  T2 tile [C; D]  C[c,r,w]=x[h0+r-1, w-1], D[c,r,w]=x[h0+r-1, w+1] (r=0..Sl+1):
    MM3-5 pair (dh,0)+(dh,2): rhs T2[:, l0+dh:l0+dh+4, :]
Every strip loads with exactly 2 dual-half DMAs (row-shifted halves for T1,
column-shifted flat windows for T2; contiguous per-partition descriptors at
full DMA bandwidth) and no fixups. A short stream of dummy warmup matmuls keeps the
PE busy (and its clock ramp going) while the first strip loads; strip
heights ramp 8..32 at the start (fast first load -> early PE start) and down
to 4 at the end (small, overlappable tail stores). PSUM->SBUF evacuation alternates between
DVE and the Activation engine so neither gates PSUM-bank recycling, and x
loads run 4 strips ahead.
"""

from contextlib import ExitStack

import numpy as np

N_CORES = 8
B, C, H, W, O = 16, 64, 112, 112, 128
B_LOC = B // N_CORES
WP = W + 1             # 112 data cols + 1 zero separator per row
FL = (H + 2) * WP + 2  # padded flat length per channel (+2 guards)

_cache = {}


def _task_list(B_loc, H):
    tasks = []
    for img in range(B_loc):
        if img == 0:
            ss = [8, 16, 24, 32, 32]   # ramp up: fast first loads
        elif img == B_loc - 1:
            ss = [28, 28, 28, 16, 8, 4]  # ramp down: small tail stores
        else:
            ss = [28, 28, 28, 28]
        h0 = 0
        for sl in ss:
            tasks.append((img, h0, sl))
            h0 += sl
        assert h0 == H
    return tasks


def _build_nc(B_loc=B_LOC, lookahead=4, n_warmup=35):
    import concourse.mybir as mybir
    import concourse.tile as tile
    from concourse import bacc
    from concourse.bass import AP

    BF16 = mybir.dt.bfloat16
    F32 = mybir.dt.float32

    nc = bacc.Bacc("TRN2", target_bir_lowering=False, debug=False,
                   num_devices=N_CORES)
    x_d = nc.declare_dram_parameter("x", [B_loc, C, FL], BF16, isOutput=False)
    w_d = nc.declare_dram_parameter("weight", [C, 3, 3, O], BF16, isOutput=False)
    o_d = nc.declare_dram_parameter("out", [B_loc, O, H, W], BF16, isOutput=True)

    tasks = _task_list(B_loc, H)
    slmax = max(t[2] for t in tasks)

    with tile.TileContext(nc) as tc, ExitStack() as ctx:
        wpool = ctx.enter_context(tc.tile_pool(name="weights", bufs=1))
        t1pool = ctx.enter_context(tc.tile_pool(name="t1", bufs=5))
        t2pool = ctx.enter_context(tc.tile_pool(name="t2", bufs=5))
        spool = ctx.enter_context(tc.tile_pool(name="staging", bufs=4))
        ppool = ctx.enter_context(tc.tile_pool(name="psum", bufs=8, space="PSUM"))

        # ---- PE warmup: dummy matmuls on zeroed scratch while loads run ----
        scr_w = wpool.tile([128, O], BF16, tag="scr_w")
        scr_x = wpool.tile([128, W], BF16, tag="scr_x")
        ps_warm = ppool.tile([O, 4, W], F32, tag="ps")
        nc.gpsimd.memset(scr_w[:, :], 0.0)
        nc.gpsimd.memset(scr_x[:, :], 0.0)
        for _ in range(n_warmup):
            nc.tensor.matmul(ps_warm[:, 0, :], scr_w[:, :], scr_x[:, :],
                             start=True, stop=True)

        # ---- weights ----
        # One DMA for the whole weight, then split into the 5 lhsT tiles
        # on-chip with DVE copies.
        wraw = wpool.tile([64, 9, O], BF16, tag="wraw")
        p01c = wpool.tile([128, O], BF16, tag="p01c")
        s21 = wpool.tile([128, O], BF16, tag="s21")
        p02 = [wpool.tile([128, O], BF16, tag=f"p02_{dh}", name=f"p02_{dh}")
               for dh in range(3)]

        def load_weights():
            # wraw[c, dh*3+dw, o] = w[c, dh, dw, o]
            nc.sync.dma_start(wraw[:, :, :], w_d[:, :, :, :])
            nc.vector.tensor_copy(p01c[0:64, :], wraw[:, 1, :])    # W(0,1)
            nc.vector.tensor_copy(p01c[64:128, :], wraw[:, 4, :])  # W(1,1)
            nc.vector.memset(s21[0:64, :], 0.0)
            nc.vector.tensor_copy(s21[64:128, :], wraw[:, 7, :])   # W(2,1)
            for dh in range(3):
                nc.vector.tensor_copy(p02[dh][0:64, :], wraw[:, 3 * dh, :])
                nc.vector.tensor_copy(p02[dh][64:128, :],
                                      wraw[:, 3 * dh + 2, :])

        def load_task(t):
            img, h0, sl = t
            r1 = sl + 1
            r2 = sl + 2
            t1 = t1pool.tile([128, slmax + 1, WP], BF16, tag="t1",
                             name="t1")[:, 0:r1, :]
            t2 = t2pool.tile([128, slmax + 2, WP], BF16, tag="t2",
                             name="t2")[:, 0:r2, :]
            t1f = t1.rearrange("p a b -> p (a b)")
            t2f = t2.rearrange("p a b -> p (a b)")
            base = img * C * FL + 1 + h0 * WP  # flat addr of padded row h0

            # T1 halves: A rows <- padded rows h0..h0+sl (= x rows h0-1..),
            #            B rows <- padded rows h0+1..h0+sl+1 (shift dim +WP)
            nc.sync.dma_start(
                t1f[:, :],
                AP(x_d, base, [[WP, 2], [FL, C], [1, r1 * WP]]))
            # T2 halves: C flat <- [base-1 : +r2*WP], D flat <- [base+1 : ..]
            # row separators provide the zero edge columns under the +-1
            # shifts -- no fixup memsets needed.
            nc.sync.dma_start(
                t2f[:, :],
                AP(x_d, base - 1, [[2, 2], [FL, C], [1, r2 * WP]]))
            return t1, t2

        def compute(t, tiles):
            img, h0, sl = t
            t1, t2 = tiles
            stg = spool.tile([O, slmax, W], BF16, tag="stg",
                             name="stg")[:, 0:sl, :]
            for j in range(sl // 4):
                l0 = 4 * j
                ps = ppool.tile([O, 4, W], F32, tag="ps")
                nc.tensor.matmul(ps[:, :, :], p01c[:, :],
                                 t1[:, l0:l0 + 4, 0:W], start=True,
                                 stop=False)
                nc.tensor.matmul(ps[:, :, :], s21[:, :],
                                 t1[:, l0 + 1:l0 + 5, 0:W], start=False,
                                 stop=False)
                for dh in range(3):
                    nc.tensor.matmul(ps[:, :, :], p02[dh][:, :],
                                     t2[:, l0 + dh:l0 + dh + 4, 0:W],
                                     start=False, stop=(dh == 2))
                if j % 2 == 0:
                    nc.vector.tensor_copy(stg[:, l0:l0 + 4, :], ps[:, :, :])
                else:
                    nc.scalar.copy(stg[:, l0:l0 + 4, :], ps[:, :, :])
            nc.sync.dma_start(o_d[img, :, h0:h0 + sl, :], stg[:, :, :])

        n = len(tasks)
        tiles = {}
        load_weights()
        tiles[0] = load_task(tasks[0])
        for i in range(1, min(lookahead, n)):
            tiles[i] = load_task(tasks[i])
        for i in range(n):
            compute(tasks[i], tiles.pop(i))
            if i + lookahead < n:
                tiles[i + lookahead] = load_task(tasks[i + lookahead])

    nc.compile()
    return nc


def _pad_x_flat(x):
    """(B, C, H, W) -> (B, C, FL): H-padded rows of W+1 elements (data ++
    one zero separator), one zero guard at each end (see module doc)."""
    Bn, Cn, Hn, Wn = x.shape
    xp = np.zeros((Bn, Cn, FL), dtype=x.dtype)
    rows = xp[:, :, 1:1 + (Hn + 2) * WP].reshape(Bn, Cn, Hn + 2, WP)
    rows[:, :, 1:1 + Hn, 0:Wn] = x
    return xp


def _make_in_maps(x, weight):
    import ml_dtypes

    bf16 = ml_dtypes.bfloat16
    xp = _pad_x_flat(np.ascontiguousarray(np.asarray(x).astype(bf16)))
    w = np.ascontiguousarray(np.asarray(weight).astype(bf16))
    return [
        {"x": xp[i * B_LOC:(i + 1) * B_LOC], "weight": w}
        for i in range(N_CORES)
    ]


def kernel(x: np.ndarray, weight: np.ndarray) -> np.ndarray:
    from concourse.bass_utils import run_bass_kernel_spmd

    if "nc" not in _cache:
        _cache["nc"] = _build_nc()
    nc = _cache["nc"]

    in_maps = _make_in_maps(x, weight)
    res = run_bass_kernel_spmd(nc, in_maps, list(range(N_CORES)))
    out = np.concatenate(
        [np.asarray(res.results[i]["out"]) for i in range(N_CORES)], axis=0
    )
    return out.astype(np.float32)
